# revision 27
# baseline (speedup 1.0000x reference)
"""CBOW negative-sampling loss kernel for Trainium2 (8 NeuronCores, Bass/Tile).

Sharding: data-parallel over the batch dim (16384 -> 8 x 2048 items), embedding
tables replicated per core.

Per core, both streams (ctx rows from W_embed, target+neg rows from W_out) are
window-sorted (int16 gather limit), gathered in f32, cast to bf16, and
scatter-added into canonical per-(item, role) SBUF slots via the parity-split
CCE path -- no DRAM staging, no pair-gather:

- ctx: slots (item, c 0..9); tree-add -> ctx_sum[item] in SBUF.
- out: slots (item, j 0..11) with j=0 target, 1..10 negatives, 11 a spare that
  stays zero; pads go to a dustbin slot excluded from the reduce.
- scores = reduce_D(out_slots * ctx_sum) per slot; a static +-0.1 sign tile
  folds the 1/CTX scale and the target sign; softplus via Exp -> Ln(1+x)
  with a per-partition accumulator.  Spare slots contribute exactly ln(2)
  each (score 0), subtracted on host.

The 2048 items are split into four quarters (4 tiles each) with separate
accumulator pairs, giving 8 independent scatter RMW chains, emitted
depth-first by quarter so only the last quarter's reduction is exposed as a
tail.  Window caps are exact per-input (max over cores, rounded to 128),
computed at kernel() time; the program cache is keyed by the cap tuple.
Gather->scatter emission lags by one round so the in-order Pool engine never
waits on a cast; softplus is Exp -> Ln(1+x) on the Act engine with warmed
tables.
"""
import sys

if '/opt/trn_rl_repo' not in sys.path:
    sys.path.insert(0, '/opt/trn_rl_repo')

import numpy as np

P = 128          # partitions
D = 128          # embedding dim
CTX = 10         # context window
NOUT = 11        # 1 target + 10 negatives
OUT_SLOTS = 12   # 11 roles + 1 spare (even slot count for parity split)
V = 100000       # vocab rows
WIN = 32768      # int16 gather window
NW = (V + WIN - 1) // WIN
NCORES = 8
RING = 1024      # max rows per gather/scatter call (desc-ring bound)
SCRATCH = 16384  # bytes/partition of SWDGE descriptor carveout (1024-desc ring)
NQ = 4           # item quarters (independent scatter chains)

_PROGRAM_CACHE = {}
LN2 = float(np.log(np.float32(2.0)))


def _round_up(x, m):
    return ((x + m - 1) // m) * m


def _chunks_of(cap):
    """Split cap into balanced 128-aligned chunks of <= RING rows."""
    if cap == 0:
        return []
    n = (cap + RING - 1) // RING
    c = _round_up((cap + n - 1) // n, P)
    out = []
    left = cap
    while left > 0:
        take = min(c, left)
        out.append(take)
        left -= take
    return out


class _Plan:
    """Static program layout for a per-core batch of T*128 items.

    caps[stream][q][w] are exact per-window stream capacities (max over the
    8 cores, rounded up to 128).
    """

    def __init__(self, T, ctx_caps, out_caps):
        assert T % NQ == 0
        self.T = T
        self.TH = T // NQ
        self.items = T * P
        self.ctx_caps = tuple(tuple(h) for h in ctx_caps)  # [q][w]
        self.out_caps = tuple(tuple(h) for h in out_caps)

        def mk_calls(caps):
            calls = []  # per quarter: list of (w, off, n); off global
            totals = []
            for h in range(NQ):
                hc = []
                off = 0
                for w in range(NW):
                    for c in _chunks_of(caps[h][w]):
                        hc.append((w, off, c))
                        off += c
                calls.append(hc)
                totals.append(off)
            return calls, totals

        self.ctx_calls, self.ctx_totals = mk_calls(self.ctx_caps)
        self.out_calls, self.out_totals = mk_calls(self.out_caps)
        # emission pairs ctx/out calls per quarter; equalize list lengths
        # by splitting the largest call of the shorter list
        for h in range(NQ):
            a, b = self.ctx_calls[h], self.out_calls[h]
            while len(a) != len(b):
                short = a if len(a) < len(b) else b
                i = max(range(len(short)), key=lambda j: short[j][2])
                w, off, n = short[i]
                n1 = _round_up(n // 2, P)
                short[i:i + 1] = [(w, off, n1), (w, off + n1, n - n1)]
        # canonical slot-rank spaces per quarter (slot = partition + P*rank)
        self.r_dust_ctx = CTX * self.TH          # even -> acc_e group
        self.g_ctx = self.r_dust_ctx // 2 + 1    # groups incl dustbin
        self.r_dust_out = OUT_SLOTS * self.TH
        self.g_out = self.r_dust_out // 2 + 1
        self.s_dust_ctx = (P - 1) + P * self.r_dust_ctx
        self.s_dust_out = (P - 1) + P * self.r_dust_out
        # every item's spare slot (j=11) stays zero -> contributes ln2
        self.spare_per_core = self.items


def _wrap_idx(vals):
    """dma_gather idx layout: idx[i] read from [i%16, i//16]; replicate to
    128 partitions so every queue's 16-partition band sees the list."""
    n = len(vals)
    assert n % 16 == 0
    arr = np.asarray(vals, np.int16).reshape(n // 16, 16).T  # [16, n/16]
    return np.tile(arr, (8, 1))                              # [128, n/16]


def _quarter_window_counts(ids, items, k):
    """ids [items, k] -> per (quarter, window) draw counts."""
    w_of = np.minimum(np.asarray(ids, np.int64) // WIN, NW - 1)
    qtr = (np.arange(items) // P // (items // P // NQ))[:, None]
    out = np.zeros((NQ, NW), np.int64)
    for h in range(NQ):
        sel = np.broadcast_to(qtr == h, w_of.shape)
        np.add.at(out[h], w_of[sel].ravel(), 1)
    return out


def _host_prep_core(plan, ctx_ids, tgt_ids, neg_ids):
    """Build the per-core gather/scatter int16 index tensors."""
    TH = plan.TH

    item_idx = np.arange(plan.items)
    tile = item_idx // P
    part = item_idx % P
    qtr = tile // TH
    ltile = tile - qtr * TH

    def build(ids, slot_of, caps, dust):
        w_of = np.minimum(ids // WIN, NW - 1)
        rel = ids - w_of * WIN
        q2 = np.broadcast_to(qtr[:, None], ids.shape)
        gs, ss = [], []
        for h in range(NQ):
            for w in range(NW):
                sel = (w_of == w) & (q2 == h)
                g = rel[sel]
                s = slot_of[sel]
                cap = caps[h][w]
                if len(g) > cap:
                    raise RuntimeError(
                        f"quarter {h} window {w} overflow: {len(g)} > {cap}")
                pad = cap - len(g)
                gs.append(np.concatenate([g, np.zeros(pad, np.int64)]))
                ss.append(np.concatenate([s, np.full(pad, dust)]))
        return (_wrap_idx(np.concatenate(gs)), _wrap_idx(np.concatenate(ss)))

    # ctx: slot rank = CTX*ltile + c
    ids = np.asarray(ctx_ids, np.int64)
    slot = part[:, None] + P * (CTX * ltile[:, None] + np.arange(CTX)[None, :])
    ctx_g, ctx_s = build(ids, slot, plan.ctx_caps, plan.s_dust_ctx)

    # out: slot rank = OUT_SLOTS*ltile + j  (j=0 target, 1..10 negatives)
    oids = np.concatenate(
        [np.asarray(tgt_ids, np.int64)[:, None],
         np.asarray(neg_ids, np.int64)], axis=1)          # [items, NOUT]
    slot = part[:, None] + P * (OUT_SLOTS * ltile[:, None]
                                + np.arange(NOUT)[None, :])
    out_g, out_s = build(oids, slot, plan.out_caps, plan.s_dust_out)

    return {
        "ctx_gidx": ctx_g, "ctx_sidx": ctx_s,
        "out_gidx": out_g, "out_sidx": out_s,
    }


def _build_program(plan, repeat=1):
    from contextlib import ExitStack

    import concourse.bacc as bacc
    import concourse.mybir as mybir
    import concourse.tile as tile
    from concourse.library_config import mlp as mlp_lib

    T = plan.T
    TH = plan.TH
    f32 = mybir.dt.float32
    bf16 = mybir.dt.bfloat16
    i16 = mybir.dt.int16
    AL = mybir.AluOpType
    AF = mybir.ActivationFunctionType
    GH = OUT_SLOTS // 2  # 6
    HC = CTX // 2        # 5

    nc = bacc.Bacc("TRN2", num_swdge_queues=1,
                   dynamic_dma_scratch_size=SCRATCH)

    ctx_tot = sum(plan.ctx_totals)
    out_tot = sum(plan.out_totals)
    w_embed = nc.dram_tensor("w_embed", (V, D), f32, kind="ExternalInput")
    w_out = nc.dram_tensor("w_out", (V, D), f32, kind="ExternalInput")
    ctx_gidx = nc.dram_tensor("ctx_gidx", (P, ctx_tot // 16), i16,
                              kind="ExternalInput")
    ctx_sidx = nc.dram_tensor("ctx_sidx", (P, ctx_tot // 16), i16,
                              kind="ExternalInput")
    out_gidx = nc.dram_tensor("out_gidx", (P, out_tot // 16), i16,
                              kind="ExternalInput")
    out_sidx = nc.dram_tensor("out_sidx", (P, out_tot // 16), i16,
                              kind="ExternalInput")
    out = nc.dram_tensor("out", (P, 1), f32, kind="ExternalOutput")

    win_starts = [w * WIN for w in range(NW)]
    win_ends = [min((w + 1) * WIN, V) for w in range(NW)]

    with tile.TileContext(nc) as tc, ExitStack() as ctx:
        cpool = ctx.enter_context(tc.tile_pool(name="const", bufs=1))
        gpool = ctx.enter_context(tc.tile_pool(name="work", bufs=4))
        kpool = ctx.enter_context(tc.tile_pool(name="cast", bufs=2))
        tpool = ctx.enter_context(tc.tile_pool(name="tree", bufs=1))

        nc.gpsimd.load_library(mlp_lib)

        # index tiles
        cg = cpool.tile([P, ctx_tot // 16], i16)
        cs = cpool.tile([P, ctx_tot // 16], i16)
        og = cpool.tile([P, out_tot // 16], i16)
        os_ = cpool.tile([P, out_tot // 16], i16)
        nc.sync.dma_start(out=cg[:], in_=ctx_gidx[:][:, :])
        nc.scalar.dma_start(out=og[:], in_=out_gidx[:][:, :])
        nc.sync.dma_start(out=cs[:], in_=ctx_sidx[:][:, :])
        nc.scalar.dma_start(out=os_[:], in_=out_sidx[:][:, :])

        # static sign tile for scores [P, q, parity, TH, slots/2]: +0.1
        # everywhere, -0.1 at parity=0 local-slot 0 (j=0 target). The 0.1
        # folds the 1/CTX context-mean scale.
        sign = cpool.tile([P, NQ, 2, TH, GH], bf16)
        nc.vector.memset(sign[:], 1.0 / CTX)
        nc.vector.memset(sign[:, :, 0:1, :, 0:1], -1.0 / CTX)

        # zero source for Act-engine broadcast zero-fills + table warmup
        warm = cpool.tile([P, 1], f32)
        nc.vector.memset(warm[:], 0.0)
        nc.scalar.activation(out=warm[:], in_=warm[:], func=AF.Exp)
        nc.scalar.activation(out=warm[:], in_=warm[:], func=AF.Ln, bias=1.0)
        nc.vector.memset(warm[:], 0.0)

        with nc.allow_low_precision("bf16 pipeline validated vs f32 ref"):
          for _rep in range(repeat):
            # accumulators per quarter: parity pairs for ctx and out
            ce = [cpool.tile([P, plan.g_ctx, D], bf16, tag=f"ce{h}",
                             name=f"ce{h}") for h in range(NQ)]
            co = [cpool.tile([P, plan.g_ctx, D], bf16, tag=f"co{h}",
                             name=f"co{h}") for h in range(NQ)]
            oe = [cpool.tile([P, plan.g_out, D], bf16, tag=f"oe{h}",
                             name=f"oe{h}") for h in range(NQ)]
            oo = [cpool.tile([P, plan.g_out, D], bf16, tag=f"oo{h}",
                             name=f"oo{h}") for h in range(NQ)]

            # all accumulators zeroed on DVE, in first-use order
            # (quarter h's scatters start at round ~2h; DVE finishes
            # zeroing quarter h by ~6(2h+2) us)
            for h in range(NQ):
                nc.vector.memset(ce[h][:], 0.0)
                nc.vector.memset(co[h][:], 0.0)
                nc.vector.memset(oe[h][:], 0.0)
                nc.vector.memset(oo[h][:], 0.0)

            seq = []  # rounds of ((0,h)+ctx_call, (1,h)+out_call)
            for h in range(NQ):
                for ca, cb in zip(plan.ctx_calls[h], plan.out_calls[h]):
                    seq.append(((0, h) + ca, (1, h) + cb))

            def emit_gather_cast(stream, qq, w, off, n):
                if stream == 0:
                    table, gi, gq = w_embed, cg, 0
                    hoff = off + sum(plan.ctx_totals[:qq])
                else:
                    table, gi, gq = w_out, og, 0
                    hoff = off + sum(plan.out_totals[:qq])
                g = n // 16
                gr = n // P
                raw = gpool.tile([P, RING // P, D], f32, tag="raw")
                nc.gpsimd.dma_gather(
                    out_ap=raw[:, :gr, :],
                    in_ap=table[win_starts[w]:win_ends[w], :],
                    idxs_ap=gi[:, hoff // 16:hoff // 16 + g],
                    num_idxs=n,
                    num_idxs_reg=n,
                    elem_size=D,
                    queue_num=gq,
                )
                cb = kpool.tile([P, RING // P, D], bf16, tag="cb")
                nc.scalar.activation(
                    out=cb[:, :gr, :], in_=raw[:, :gr, :], func=AF.Copy)
                return cb, gr, hoff

            def emit_scatter(stream, qq, w, off, n, cb, gr, hoff):
                if stream == 0:
                    si, sq = cs, 0
                    eacc, oacc = ce[qq], co[qq]
                else:
                    si, sq = os_, 0
                    eacc, oacc = oe[qq], oo[qq]
                g = n // 16
                nc.gpsimd.dma_scatter_add(
                    out_ap=eacc[:],
                    in_ap=cb[:, :gr, :],
                    idxs_ap=si[:, hoff // 16:hoff // 16 + g],
                    num_idxs=n,
                    num_idxs_reg=n,
                    elem_size=D,
                    queue_num=sq,
                    sbuf_tokens_per_rank=P,
                    parity_reg=0,
                    out_ap_other=oacc[:],
                )

            pend = []
            for k, (ctx_call, out_call) in enumerate(seq):
                done = []
                for call in (ctx_call, out_call):
                    stream, qq, w, off, n = call
                    cb, gr, hoff = emit_gather_cast(stream, qq, w, off, n)
                    done.append(call + (cb, gr, hoff))
                pend.append(done)
                if len(pend) > 1:
                    for args in pend.pop(0):
                        emit_scatter(*args)
            while pend:
                for args in pend.pop(0):
                    emit_scatter(*args)

            # ---- per-quarter reduction ----
            united = cpool.tile([P, T, D], bf16, tag="united")
            scr = cpool.tile([P, NQ, 2, TH, GH], bf16, tag="scr")
            for h in range(NQ):
                # ctx sum: rank r = CTX*lt + c; even c in ce at
                # g = HC*lt + c//2, odd c in co likewise.  Sequential adds
                # through two small aux tiles.
                ve = ce[h][:, :TH * HC, :].rearrange(
                    "p (t k) d -> p t k d", k=HC)
                vo = co[h][:, :TH * HC, :].rearrange(
                    "p (t k) d -> p t k d", k=HC)
                ae = tpool.tile([P, TH, 1, D], bf16, tag="ae", name="ae")
                nc.vector.tensor_tensor(
                    out=ae[:], in0=ve[:, :, 0:1, :], in1=ve[:, :, 1:2, :],
                    op=AL.add)
                ao = tpool.tile([P, TH, 1, D], bf16, tag="ao", name="ao")
                nc.vector.tensor_tensor(
                    out=ao[:], in0=vo[:, :, 0:1, :], in1=vo[:, :, 1:2, :],
                    op=AL.add)
                for k in range(2, HC):
                    nc.vector.tensor_tensor(
                        out=ae[:], in0=ae[:], in1=ve[:, :, k:k + 1, :],
                        op=AL.add)
                    nc.vector.tensor_tensor(
                        out=ao[:], in0=ao[:], in1=vo[:, :, k:k + 1, :],
                        op=AL.add)
                uh = united[:, h * TH:(h + 1) * TH, :]
                nc.vector.tensor_tensor(
                    out=uh, in0=ae[:, :, 0, :], in1=ao[:, :, 0, :], op=AL.add)

                # scores: per-slot product+reduce with regular APs
                se = oe[h][:, :TH * GH, :].rearrange(
                    "p (t g) d -> p t g d", g=GH)
                so = oo[h][:, :TH * GH, :].rearrange(
                    "p (t g) d -> p t g d", g=GH)
                uh4 = united[:, h * TH:(h + 1) * TH, :].unsqueeze(2)
                pr = tpool.tile([P, TH, 1, D], bf16, tag="pr", name="pr")
                for g in range(GH):
                    nc.vector.tensor_tensor(
                        out=pr[:], in0=se[:, :, g:g + 1, :], in1=uh4,
                        op=AL.mult)
                    nc.vector.tensor_reduce(
                        out=scr[:, h, 0, :, g:g + 1], in_=pr[:],
                        axis=mybir.AxisListType.X, op=AL.add)
                    nc.vector.tensor_tensor(
                        out=pr[:], in0=so[:, :, g:g + 1, :], in1=uh4,
                        op=AL.mult)
                    nc.vector.tensor_reduce(
                        out=scr[:, h, 1, :, g:g + 1], in_=pr[:],
                        axis=mybir.AxisListType.X, op=AL.add)

            # ---- softplus(sign * score / CTX) + accumulate ----
            nc.vector.tensor_tensor(
                out=scr[:], in0=scr[:], in1=sign[:], op=AL.mult)
            es = cpool.tile([P, NQ, 2, TH, GH], bf16, tag="es")
            nc.scalar.activation(out=es[:], in_=scr[:], func=AF.Exp)
            ls = cpool.tile([P, NQ, 2, TH, GH], bf16, tag="ls")
            acc = cpool.tile([P, 1], f32, tag="acc")
            nc.scalar.activation(
                out=ls[:], in_=es[:], func=AF.Ln, bias=1.0, accum_out=acc[:])
            nc.sync.dma_start(out=out[:][:, :], in_=acc[:])

    if not nc.is_finalized():
        nc.finalize()
    return nc


def _get_program(plan, repeat=1):
    key = (plan.T, plan.ctx_caps, plan.out_caps, repeat)
    if key not in _PROGRAM_CACHE:
        _PROGRAM_CACHE[key] = _build_program(plan, repeat=repeat)
    return _PROGRAM_CACHE[key]


def _make_plan(context_ids, target_ids, neg_ids):
    B = context_ids.shape[0]
    assert B % (NCORES * P) == 0, B
    T = B // (NCORES * P)
    items = T * P
    ctx = np.asarray(context_ids).reshape(NCORES, items, CTX)
    oid = np.concatenate(
        [np.asarray(target_ids).reshape(NCORES, items, 1),
         np.asarray(neg_ids).reshape(NCORES, items, NOUT - 1)], axis=2)
    ctx_caps = np.zeros((NQ, NW), np.int64)
    out_caps = np.zeros((NQ, NW), np.int64)
    for c in range(NCORES):
        ctx_caps = np.maximum(
            ctx_caps, _quarter_window_counts(ctx[c], items, CTX))
        out_caps = np.maximum(
            out_caps, _quarter_window_counts(oid[c], items, NOUT))
    ctx_caps = [[int(_round_up(n, P)) for n in h] for h in ctx_caps]
    out_caps = [[int(_round_up(n, P)) for n in h] for h in out_caps]
    return _Plan(T, ctx_caps, out_caps), B, T


def _prep_inputs(W_embed, W_out, context_ids, target_ids, neg_ids):
    plan, B, T = _make_plan(context_ids, target_ids, neg_ids)

    w_e = np.ascontiguousarray(np.asarray(W_embed, dtype=np.float32))
    w_o = np.ascontiguousarray(np.asarray(W_out, dtype=np.float32))
    ctx = np.asarray(context_ids).reshape(NCORES, plan.items, CTX)
    tgt = np.asarray(target_ids).reshape(NCORES, plan.items)
    neg = np.asarray(neg_ids).reshape(NCORES, plan.items, NOUT - 1)

    in_maps = []
    for c in range(NCORES):
        m = _host_prep_core(plan, ctx[c], tgt[c], neg[c])
        m["w_embed"] = w_e
        m["w_out"] = w_o
        in_maps.append(m)
    return in_maps, B, T, plan


def _run(W_embed, W_out, context_ids, target_ids, neg_ids, **spmd_kwargs):
    from concourse import bass_utils

    in_maps, B, T, plan = _prep_inputs(
        W_embed, W_out, context_ids, target_ids, neg_ids)
    nc = _get_program(plan)
    res = bass_utils.run_bass_kernel_spmd(
        nc, in_maps, core_ids=list(range(NCORES)), **spmd_kwargs)
    total = 0.0
    for r in res.results:
        total += float(r["out"].astype(np.float64).sum())
    total -= NCORES * plan.spare_per_core * LN2
    loss = np.float32(total / B)
    return loss, res


def kernel(W_embed, W_out, context_ids, target_ids, neg_ids):
    loss, _ = _run(W_embed, W_out, context_ids, target_ids, neg_ids)
    return loss
